# revision 18
# baseline (speedup 1.0000x reference)
"""Multi-headed self-attention (B=2, S=2048, D=1024, H=16) on 8 TRN2 cores.

Sharding: hybrid batch x head tensor-parallel. Core c handles batch c//4 and
heads (c%4)*4 .. (c%4)*4+3. Each core computes x = query[b] + pos_emb, the QKV
projection for its 4 heads, attention, and a partial output projection
(o_heads @ w_out_rows). Host sums the 4 partials per batch.

v2 notes (vs the f32r v1 baseline):
- Inputs (queryT, pos_embT, w_qkv) ship in bf16: halves the input DMA and the
  QKV projection runs as bf16 matmuls (same PE rate as f32r, exact f32 psum).
- The exp over attention scores is split between the Scalar engine (true exp)
  and the Vector engine (Schraudolph-style 2^y bit trick: i16 = s*C + B,
  bitcast to bf16), so the PE's QK->AV pipeline is never gated on a single
  activation engine. DVE handles DVE_KBS of the 16 key blocks per head.
- Attention weights (ptile) and V are bf16; QK^T consumes bf16 q/k. All psum
  accumulation stays f32.
- Softmax denominators come from a ones-column in the AV matmul; reciprocal is
  applied to the single denominator row, then broadcast to 64 partitions via a
  DRAM-staged partition-broadcast DMA (bandwidth is idle during attention).
- Output projection keeps f32r (oT x w_out), result stored bf16 (halves the
  output DMA); the host upcasts and sums partials in f32.
"""

import os
import sys

import numpy as np

if "/opt/trn_rl_repo" not in sys.path:
    sys.path.insert(0, "/opt/trn_rl_repo")

B, S, D, H = 2, 2048, 1024, 16
DK = 64
P = 128
NCORES = 8
HPC = H // (NCORES // B)  # heads per core = 4
T = S  # tokens per core (one batch)
E = HPC * 3 * DK  # 768 qkv output columns per core
NDC = D // P  # 8 contraction chunks
NEC = E // P  # 6 projection output chunks
NTB = T // P  # 16 token blocks
NTG = T // 512  # 4 token groups of 512
SCALE = DK**-0.5

# Schraudolph bf16-bits exp: exp(s*SCALE) ~ bf16(int16(s*EXP_C + EXP_B))
EXP_C = float(SCALE * 128 * np.log2(np.e))
EXP_B = 16249.1
# key blocks (of 16 per head) whose exp runs on DVE instead of ACT
DVE_SET = (2, 6, 10, 14)
# timing probe: 1 = skip exp instructions (AV consumes garbage), 2 = skip AV
PROBE_SKIP = int(os.environ.get("PROBE_SKIP", "0"))

_CACHE = {}


def _build_program(reps=1):
    from contextlib import ExitStack, nullcontext

    import concourse.bass as bass
    import concourse.tile as tile
    from concourse import bacc
    from concourse import mybir
    from concourse.masks import make_identity

    f32 = mybir.dt.float32
    f32r = mybir.dt.float32r
    bf16 = mybir.dt.bfloat16
    i16 = mybir.dt.int16
    EXP = mybir.ActivationFunctionType.Exp
    MULT = mybir.AluOpType.mult
    ADD = mybir.AluOpType.add

    nc = bacc.Bacc()
    xqT = nc.declare_dram_parameter("xqT", [D, T], bf16, isOutput=False)
    posT = nc.declare_dram_parameter("posT", [D, T], bf16, isOutput=False)
    wqkv = nc.declare_dram_parameter("wqkv", [D, E], bf16, isOutput=False)
    wout = nc.declare_dram_parameter("wout", [HPC * DK, D], f32, isOutput=False)
    out = nc.declare_dram_parameter("out", [T, D], bf16, isOutput=True)

    with tile.TileContext(nc) as tc, ExitStack() as top:
        const = top.enter_context(tc.tile_pool(name="const", bufs=1))
        w_sb = const.tile([P, NDC, E], bf16)
        wout_sb = const.tile([P, 2, D], f32r)
        ident = const.tile([P, P], bf16)
        make_identity(nc, ident[:])
        qkvT = const.tile([P, 2, T], bf16)  # q01, q23 (feature-major)
        # Per-head k, zero-padded to the full 128 partitions: rows (h%2)*64..+64
        # hold k_h, the other 64 rows stay zero. QK^T then contracts over 128
        # partitions (the pad rows kill the co-resident head's q in the moving
        # operand), keeping the PE in the same (128,128) tile config as every
        # other matmul in the kernel -- no config switching in the kb loop.
        kTp = const.tile([P, HPC, T], bf16)
        V_sb = const.tile([P, NTB, HPC, DK + 2], bf16)  # col DK = ones
        oT = const.tile([P, 2, T], f32r)  # per-head-pair attention outputs

        for h in range(HPC):
            nc.gpsimd.memset(V_sb[:, :, h, DK : DK + 1], 1.0)
            pad0 = (1 - h % 2) * DK
            nc.gpsimd.memset(kTp[pad0 : pad0 + DK, h, :], 0.0)

        # reps>1 wraps the body in an on-device loop (timing builds only)
        rep_ctx = tc.For_i(0, reps, 1) if reps > 1 else nullcontext()
        top.enter_context(rep_ctx)

        # ---- Phase 1: x = queryT + pos_embT (bf16), qkv projection ----
        with (
            tc.tile_pool(name="xt", bufs=3) as xt_pool,
            tc.tile_pool(name="ldt", bufs=8) as ld_pool,
            tc.tile_pool(name="vt", bufs=3) as vt_pool,
            tc.tile_pool(name="psp", bufs=6, space="PSUM") as psum_p,
            tc.tile_pool(name="pst", bufs=2, space="PSUM") as psum_t,
        ):
            for tg in range(NTG):
                c0 = tg * 512
                xts = xt_pool.tile([P, NDC, 512], bf16, name="xts", tag="xt")
                ps = [
                    psum_p.tile([P, 512], f32, name=f"psp{ec}", tag="psp")
                    for ec in range(NEC)
                ]
                for dc in range(NDC):
                    if tg == 0:
                        nc.sync.dma_start(w_sb[:, dc, :], wqkv[dc * P : (dc + 1) * P, :])
                    qt = ld_pool.tile([P, 512], bf16, tag="ldtmp", name="qt")
                    nc.sync.dma_start(qt[:], xqT[dc * P : (dc + 1) * P, c0 : c0 + 512])
                    pt = ld_pool.tile([P, 512], bf16, tag="ldtmp", name="pt")
                    nc.sync.dma_start(pt[:], posT[dc * P : (dc + 1) * P, c0 : c0 + 512])
                    nc.vector.tensor_add(xts[:, dc, :], qt[:], pt[:])
                    for ec in range(NEC):
                        nc.tensor.matmul(
                            ps[ec][:],
                            w_sb[:, dc, ec * P : (ec + 1) * P],
                            xts[:, dc, :],
                            start=(dc == 0),
                            stop=(dc == NDC - 1),
                        )
                # q01,q23 -> qkvT; k01,k23 -> per-head padded kTp slots;
                # v01,v23 -> transpose into V_sb
                for ec in range(2):
                    nc.any.tensor_copy(qkvT[:, ec, c0 : c0 + 512], ps[ec][:])
                for ec in range(2, 4):
                    for half in range(2):
                        h = (ec - 2) * 2 + half
                        r0 = half * DK
                        nc.any.tensor_copy(
                            kTp[r0 : r0 + DK, h, c0 : c0 + 512],
                            ps[ec][r0 : r0 + DK, :],
                        )
                for evc in range(2):
                    vtmp = vt_pool.tile([P, 512], bf16, tag="vtmp", name="vtmp")
                    nc.any.tensor_copy(vtmp[:], ps[4 + evc][:])
                    for i in range(4):
                        tb = tg * 4 + i
                        pst = psum_t.tile([P, P], bf16, tag="pst", name="pst")
                        nc.tensor.transpose(
                            pst[:], vtmp[:, i * P : (i + 1) * P], ident[:]
                        )
                        nc.any.tensor_copy(
                            V_sb[:, tb, 2 * evc : 2 * evc + 2, 0:DK],
                            pst.rearrange("p (h d) -> p h d", h=2),
                        )

        # ---- Phase 2: attention per head ----
        with (
            tc.tile_pool(name="ptl", bufs=6) as pt_pool,
            tc.tile_pool(name="ou", bufs=2) as ou_pool,
            tc.tile_pool(name="rb", bufs=2) as rb_pool,
            tc.tile_pool(name="sdp", bufs=2, space="DRAM") as dram_pool,
            tc.tile_pool(name="psqk", bufs=2, space="PSUM") as psum_qk,
            tc.tile_pool(name="psav", bufs=1, space="PSUM") as psum_av,
        ):
            for h in range(HPC):
                ecq, row = h // 2, (h % 2) * DK
                qT = qkvT[:, ecq, :]  # full 128 rows; pad rows in kTp zero them
                kT = kTp[:, h, :]
                poT = psum_av.tile([DK + 1, T], f32)  # row DK = denominators

                def emit_qk(kb, qT=qT, kT=kT):
                    # exp(scores^T * scale) for key block kb over all queries
                    ptile = pt_pool.tile([P, T], bf16, name="ptile", tag="pt")
                    for qh in range(2):
                        pqk = psum_qk.tile([P, 1024], f32, name="pqk", tag="pqk")
                        for qq in range(2):
                            q0 = qh * 1024 + qq * 512
                            nc.tensor.matmul(
                                pqk[:, qq * 512 : (qq + 1) * 512],
                                kT[:, kb * P : (kb + 1) * P],
                                qT[:, q0 : q0 + 512],
                                start=True,
                                stop=True,
                            )
                        dst = ptile[:, qh * 1024 : (qh + 1) * 1024]
                        if PROBE_SKIP == 1:
                            if qh == 0:
                                nc.vector.memset(ptile[:, 0:16], 1.0)
                        elif kb in DVE_SET:
                            nc.vector.tensor_scalar(
                                dst.bitcast(i16), pqk[:], EXP_C, EXP_B, MULT, ADD
                            )
                        else:
                            nc.scalar.activation(dst, pqk[:], EXP, scale=SCALE)
                    return ptile

                # software pipeline: QK(kb+1) issues on PE while ACT/DVE exps
                # kb's scores, so exp latency stays off the PE critical path
                ptile_cur = emit_qk(0)
                for kb in range(NTB):
                    ptile_next = emit_qk(kb + 1) if kb + 1 < NTB else None
                    for qg in range(NTG):
                        nc.tensor.matmul(
                            poT[:, qg * 512 : (qg + 1) * 512],
                            V_sb[:, kb, h, 0 : DK + 1],
                            ptile_cur[:, qg * 512 : (qg + 1) * 512],
                            start=(kb == 0),
                            stop=(kb == NTB - 1),
                        )
                    ptile_cur = ptile_next

                # Normalize: copy frees the AV psum; reciprocal on the single
                # denominator row, then broadcast to 64 partitions via DRAM
                # (attention leaves DMA bandwidth idle). For the last head the
                # whole chain is chunked per token group so the output
                # projection starts ~3us (not ~10us) after the last AV matmul.
                o_us = ou_pool.tile([DK + 1, T], f32, tag="ous")
                s_dram = dram_pool.tile([1, T], f32, name="sdram", tag="sd")
                rbc = rb_pool.tile([DK, T], f32, tag="rbc")
                chunks = NTG if h == HPC - 1 else 1
                csz = T // chunks
                # The whole chain stays on DVE: offloading the copy to ACT or
                # the muls to GpSimd both measured slower on HW (the chain is
                # latency-critical at head boundaries; cross-engine hops and
                # queueing behind ACT's exps cost more than DVE's drain).
                for cc in range(chunks):
                    sl = slice(cc * csz, (cc + 1) * csz)
                    # split the psum-freeing copy across DVE and ACT: halves
                    # each engine's drain exposure and releases poT for the
                    # next head's AV accumulation ~2us earlier
                    half = csz // 2
                    sa = slice(cc * csz, cc * csz + half)
                    sb = slice(cc * csz + half, (cc + 1) * csz)
                    nc.vector.tensor_copy(o_us[:, sa], poT[:, sa])
                    nc.scalar.copy(o_us[:, sb], poT[:, sb])
                    nc.vector.reciprocal(o_us[DK : DK + 1, sl], o_us[DK : DK + 1, sl])
                    nc.sync.dma_start(s_dram[:, sl], o_us[DK : DK + 1, sl])
                    nc.sync.dma_start(rbc[:, sl], s_dram[:, sl].partition_broadcast(DK))
                    for qg in range(csz // 512):
                        ql = slice(cc * csz + qg * 512, cc * csz + (qg + 1) * 512)
                        nc.vector.tensor_mul(
                            oT[row : row + DK, ecq, ql], o_us[0:DK, ql], rbc[:, ql]
                        )

        # ---- Phase 3: partial output projection ----
        nc.sync.dma_start(
            wout_sb[:], wout.rearrange("(c p) n -> p c n", p=P).bitcast(f32r)
        )
        with (
            tc.tile_pool(name="pso", bufs=2, space="PSUM") as psum_o,
            tc.tile_pool(name="osb", bufs=3) as osb_pool,
        ):
            for tb in range(NTB):
                po = psum_o.tile([P, D], f32)
                for pair in range(2):
                    for nh in range(2):
                        nc.tensor.matmul(
                            po[:, nh * 512 : (nh + 1) * 512],
                            oT[:, pair, tb * P : (tb + 1) * P],
                            wout_sb[:, pair, nh * 512 : (nh + 1) * 512],
                            start=(pair == 0),
                            stop=(pair == 1),
                        )
                ob = osb_pool.tile([P, D], bf16)
                nc.any.tensor_copy(ob[:], po[:])
                nc.sync.dma_start(out[tb * P : (tb + 1) * P, :], ob[:])

    nc.compile()
    return nc


def get_program():
    if "nc" not in _CACHE:
        _CACHE["nc"] = _build_program()
    return _CACHE["nc"]


def make_in_maps(query, pos_emb, w_qkv, w_out):
    import ml_dtypes

    bf16 = ml_dtypes.bfloat16
    query = np.asarray(query, dtype=np.float32)
    pos_emb = np.asarray(pos_emb, dtype=np.float32)
    w_qkv = np.asarray(w_qkv, dtype=np.float32)
    w_out = np.asarray(w_out, dtype=np.float32)
    posT = np.ascontiguousarray(pos_emb.T).astype(bf16)
    in_maps = []
    for c in range(NCORES):
        b, hb = c // (NCORES // B), (c % (NCORES // B)) * HPC
        heads = range(hb, hb + HPC)
        # w_qkv column e for head h, kind j (q/k/v), dim d: e = h*3*DK + j*DK + d
        wq_c = np.concatenate(
            [w_qkv[:, h * 3 * DK + j * DK : h * 3 * DK + (j + 1) * DK] for j in range(3) for h in heads],
            axis=1,
        )
        wout_c = np.concatenate([w_out[h * DK : (h + 1) * DK, :] for h in heads], axis=0)
        in_maps.append(
            {
                "xqT": np.ascontiguousarray(query[b].T).astype(bf16),
                "posT": posT,
                "wqkv": np.ascontiguousarray(wq_c).astype(bf16),
                "wout": np.ascontiguousarray(wout_c),
            }
        )
    return in_maps


def gather_output(results):
    out = np.zeros((B, S, D), dtype=np.float32)
    for c in range(NCORES):
        out[c // (NCORES // B)] += np.asarray(results[c]["out"], dtype=np.float32)
    return out


def kernel(query, pos_emb, w_qkv, w_out):
    from concourse.bass_utils import run_bass_kernel_spmd

    nc = get_program()
    in_maps = make_in_maps(query, pos_emb, w_qkv, w_out)
    res = run_bass_kernel_spmd(nc, in_maps, list(range(NCORES)))
    return gather_output(res.results)


# revision 20
# speedup vs baseline: 1.0548x; 1.0548x over previous
"""Multi-headed self-attention (B=2, S=2048, D=1024, H=16) on 8 TRN2 cores.

Sharding: hybrid batch x head tensor-parallel. Core c handles batch c//4 and
heads (c%4)*4 .. (c%4)*4+3. Each core computes x = query[b] + pos_emb, the QKV
projection for its 4 heads, attention, and a partial output projection
(o_heads @ w_out_rows). Host sums the 4 partials per batch.

v2 notes (vs the f32r v1 baseline):
- Inputs (queryT, pos_embT, w_qkv) ship in bf16: halves the input DMA and the
  QKV projection runs as bf16 matmuls (same PE rate as f32r, exact f32 psum).
- The exp over attention scores is split between the Scalar engine (true exp)
  and the Vector engine (Schraudolph-style 2^y bit trick: i16 = s*C + B,
  bitcast to bf16), so the PE's QK->AV pipeline is never gated on a single
  activation engine. DVE handles DVE_KBS of the 16 key blocks per head.
- Attention weights (ptile) and V are bf16; QK^T consumes bf16 q/k. All psum
  accumulation stays f32.
- Softmax denominators come from a ones-column in the AV matmul; reciprocal is
  applied to the single denominator row, then broadcast to 64 partitions via a
  DRAM-staged partition-broadcast DMA (bandwidth is idle during attention).
- Output projection keeps f32r (oT x w_out), result stored bf16 (halves the
  output DMA); the host upcasts and sums partials in f32.
"""

import os
import sys

import numpy as np

if "/opt/trn_rl_repo" not in sys.path:
    sys.path.insert(0, "/opt/trn_rl_repo")

B, S, D, H = 2, 2048, 1024, 16
DK = 64
P = 128
NCORES = 8
HPC = H // (NCORES // B)  # heads per core = 4
T = S  # tokens per core (one batch)
E = HPC * 3 * DK  # 768 qkv output columns per core
NDC = D // P  # 8 contraction chunks
NEC = E // P  # 6 projection output chunks
NTB = T // P  # 16 token blocks
NTG = T // 512  # 4 token groups of 512
SCALE = DK**-0.5

# Schraudolph bf16-bits exp: exp(s*SCALE) ~ bf16(int16(s*EXP_C + EXP_B))
EXP_C = float(SCALE * 128 * np.log2(np.e))
EXP_B = 16249.1
# key blocks (of 16 per head) whose exp runs on DVE instead of ACT
DVE_SET = (2, 6, 10, 14)
# timing probe: 1 = skip exp instructions (AV consumes garbage), 2 = skip AV
PROBE_SKIP = int(os.environ.get("PROBE_SKIP", "0"))

_CACHE = {}


def _build_program(reps=1):
    from contextlib import ExitStack, nullcontext

    import concourse.bass as bass
    import concourse.tile as tile
    from concourse import bacc
    from concourse import mybir
    from concourse.masks import make_identity

    f32 = mybir.dt.float32
    f32r = mybir.dt.float32r
    bf16 = mybir.dt.bfloat16
    i16 = mybir.dt.int16
    EXP = mybir.ActivationFunctionType.Exp
    MULT = mybir.AluOpType.mult
    ADD = mybir.AluOpType.add

    nc = bacc.Bacc()
    xqT = nc.declare_dram_parameter("xqT", [D, T], bf16, isOutput=False)
    posT = nc.declare_dram_parameter("posT", [D, T], bf16, isOutput=False)
    wqkv = nc.declare_dram_parameter("wqkv", [D, E], bf16, isOutput=False)
    wout = nc.declare_dram_parameter("wout", [HPC * DK, D], f32, isOutput=False)
    out = nc.declare_dram_parameter("out", [T, D], bf16, isOutput=True)

    with tile.TileContext(nc) as tc, ExitStack() as top:
        const = top.enter_context(tc.tile_pool(name="const", bufs=1))
        w_sb = const.tile([P, NDC, E], bf16)
        wout_sb = const.tile([P, 2, D], f32r)
        ident = const.tile([P, P], bf16)
        make_identity(nc, ident[:])
        qkvT = const.tile([P, 2, T], bf16)  # q01, q23 (feature-major)
        # Per-head k, zero-padded to the full 128 partitions: rows (h%2)*64..+64
        # hold k_h, the other 64 rows stay zero. QK^T then contracts over 128
        # partitions (the pad rows kill the co-resident head's q in the moving
        # operand), keeping the PE in the same (128,128) tile config as every
        # other matmul in the kernel -- no config switching in the kb loop.
        kTp = const.tile([P, HPC, T], bf16)
        V_sb = const.tile([P, NTB, HPC, DK + 2], bf16)  # col DK = ones
        oT = const.tile([P, 2, T], f32r)  # per-head-pair attention outputs

        for h in range(HPC):
            nc.gpsimd.memset(V_sb[:, :, h, DK : DK + 1], 1.0)
            pad0 = (1 - h % 2) * DK
            nc.gpsimd.memset(kTp[pad0 : pad0 + DK, h, :], 0.0)

        # reps>1 wraps the body in an on-device loop (timing builds only)
        rep_ctx = tc.For_i(0, reps, 1) if reps > 1 else nullcontext()
        top.enter_context(rep_ctx)

        # ---- Phase 1: x = queryT + pos_embT (bf16), qkv projection ----
        with (
            tc.tile_pool(name="xt", bufs=3) as xt_pool,
            tc.tile_pool(name="ldt", bufs=8) as ld_pool,
            tc.tile_pool(name="vt", bufs=3) as vt_pool,
            tc.tile_pool(name="psp", bufs=6, space="PSUM") as psum_p,
            tc.tile_pool(name="pst", bufs=2, space="PSUM") as psum_t,
        ):
            for tg in range(NTG):
                c0 = tg * 512
                xts = xt_pool.tile([P, NDC, 512], bf16, name="xts", tag="xt")
                ps = [
                    psum_p.tile([P, 512], f32, name=f"psp{ec}", tag="psp")
                    for ec in range(NEC)
                ]
                for dc in range(NDC):
                    if tg == 0:
                        nc.sync.dma_start(w_sb[:, dc, :], wqkv[dc * P : (dc + 1) * P, :])
                    qt = ld_pool.tile([P, 512], bf16, tag="ldtmp", name="qt")
                    nc.sync.dma_start(qt[:], xqT[dc * P : (dc + 1) * P, c0 : c0 + 512])
                    pt = ld_pool.tile([P, 512], bf16, tag="ldtmp", name="pt")
                    nc.sync.dma_start(pt[:], posT[dc * P : (dc + 1) * P, c0 : c0 + 512])
                    nc.vector.tensor_add(xts[:, dc, :], qt[:], pt[:])
                    for ec in range(NEC):
                        nc.tensor.matmul(
                            ps[ec][:],
                            w_sb[:, dc, ec * P : (ec + 1) * P],
                            xts[:, dc, :],
                            start=(dc == 0),
                            stop=(dc == NDC - 1),
                        )
                # q01,q23 -> qkvT; k01,k23 -> per-head padded kTp slots;
                # v01,v23 -> transpose into V_sb
                for ec in range(2):
                    nc.any.tensor_copy(qkvT[:, ec, c0 : c0 + 512], ps[ec][:])
                for ec in range(2, 4):
                    for half in range(2):
                        h = (ec - 2) * 2 + half
                        r0 = half * DK
                        nc.any.tensor_copy(
                            kTp[r0 : r0 + DK, h, c0 : c0 + 512],
                            ps[ec][r0 : r0 + DK, :],
                        )
                for evc in range(2):
                    vtmp = vt_pool.tile([P, 512], bf16, tag="vtmp", name="vtmp")
                    nc.any.tensor_copy(vtmp[:], ps[4 + evc][:])
                    for i in range(4):
                        tb = tg * 4 + i
                        pst = psum_t.tile([P, P], bf16, tag="pst", name="pst")
                        nc.tensor.transpose(
                            pst[:], vtmp[:, i * P : (i + 1) * P], ident[:]
                        )
                        nc.any.tensor_copy(
                            V_sb[:, tb, 2 * evc : 2 * evc + 2, 0:DK],
                            pst.rearrange("p (h d) -> p h d", h=2),
                        )

        # ---- Phase 2: attention per head ----
        with (
            tc.tile_pool(name="ptl", bufs=4) as pt_pool,
            tc.tile_pool(name="ou", bufs=2) as ou_pool,
            tc.tile_pool(name="rb", bufs=2) as rb_pool,
            tc.tile_pool(name="sdp", bufs=2, space="DRAM") as dram_pool,
            tc.tile_pool(name="psqk", bufs=2, space="PSUM") as psum_qk,
            tc.tile_pool(name="psav", bufs=1, space="PSUM") as psum_av,
        ):
            for h in range(HPC):
                ecq, row = h // 2, (h % 2) * DK
                qT = qkvT[:, ecq, :]  # full 128 rows; pad rows in kTp zero them
                kT = kTp[:, h, :]
                poT = psum_av.tile([DK + 1, T], f32)  # row DK = denominators

                def emit_qk(kb, qT=qT, kT=kT):
                    # exp(scores^T * scale) for key block kb over all queries
                    ptile = pt_pool.tile([P, T], bf16, name="ptile", tag="pt")
                    for qh in range(2):
                        pqk = psum_qk.tile([P, 1024], f32, name="pqk", tag="pqk")
                        for qq in range(2):
                            q0 = qh * 1024 + qq * 512
                            nc.tensor.matmul(
                                pqk[:, qq * 512 : (qq + 1) * 512],
                                kT[:, kb * P : (kb + 1) * P],
                                qT[:, q0 : q0 + 512],
                                start=True,
                                stop=True,
                            )
                        dst = ptile[:, qh * 1024 : (qh + 1) * 1024]
                        if PROBE_SKIP == 1:
                            if qh == 0:
                                nc.vector.memset(ptile[:, 0:16], 1.0)
                        elif kb in DVE_SET:
                            nc.vector.tensor_scalar(
                                dst.bitcast(i16), pqk[:], EXP_C, EXP_B, MULT, ADD
                            )
                        else:
                            nc.scalar.activation(dst, pqk[:], EXP, scale=SCALE)
                    return ptile

                # software pipeline: QK(kb+1) issues on PE while ACT/DVE exps
                # kb's scores, so exp latency stays off the PE critical path
                ptile_cur = emit_qk(0)
                for kb in range(NTB):
                    ptile_next = emit_qk(kb + 1) if kb + 1 < NTB else None
                    for qg in range(NTG):
                        nc.tensor.matmul(
                            poT[:, qg * 512 : (qg + 1) * 512],
                            V_sb[:, kb, h, 0 : DK + 1],
                            ptile_cur[:, qg * 512 : (qg + 1) * 512],
                            start=(kb == 0),
                            stop=(kb == NTB - 1),
                        )
                    ptile_cur = ptile_next

                # Normalize: copy frees the AV psum; reciprocal on the single
                # denominator row, then broadcast to 64 partitions via DRAM
                # (attention leaves DMA bandwidth idle). For the last head the
                # whole chain is chunked per token group so the output
                # projection starts ~3us (not ~10us) after the last AV matmul.
                o_us = ou_pool.tile([DK + 1, T], f32, tag="ous")
                s_dram = dram_pool.tile([1, T], f32, name="sdram", tag="sd")
                rbc = rb_pool.tile([DK, T], f32, tag="rbc")
                chunks = NTG if h == HPC - 1 else 1
                csz = T // chunks
                # The whole chain stays on DVE: offloading the copy to ACT or
                # the muls to GpSimd both measured slower on HW (the chain is
                # latency-critical at head boundaries; cross-engine hops and
                # queueing behind ACT's exps cost more than DVE's drain).
                for cc in range(chunks):
                    sl = slice(cc * csz, (cc + 1) * csz)
                    nc.vector.tensor_copy(o_us[:, sl], poT[:, sl])
                    nc.vector.reciprocal(o_us[DK : DK + 1, sl], o_us[DK : DK + 1, sl])
                    nc.sync.dma_start(s_dram[:, sl], o_us[DK : DK + 1, sl])
                    nc.sync.dma_start(rbc[:, sl], s_dram[:, sl].partition_broadcast(DK))
                    for qg in range(csz // 512):
                        ql = slice(cc * csz + qg * 512, cc * csz + (qg + 1) * 512)
                        nc.vector.tensor_mul(
                            oT[row : row + DK, ecq, ql], o_us[0:DK, ql], rbc[:, ql]
                        )

        # ---- Phase 3: partial output projection ----
        nc.sync.dma_start(
            wout_sb[:], wout.rearrange("(c p) n -> p c n", p=P).bitcast(f32r)
        )
        with (
            tc.tile_pool(name="pso", bufs=2, space="PSUM") as psum_o,
            tc.tile_pool(name="osb", bufs=3) as osb_pool,
        ):
            for tb in range(NTB):
                po = psum_o.tile([P, D], f32)
                for pair in range(2):
                    for nh in range(2):
                        nc.tensor.matmul(
                            po[:, nh * 512 : (nh + 1) * 512],
                            oT[:, pair, tb * P : (tb + 1) * P],
                            wout_sb[:, pair, nh * 512 : (nh + 1) * 512],
                            start=(pair == 0),
                            stop=(pair == 1),
                        )
                ob = osb_pool.tile([P, D], bf16)
                # alternate the psum-draining copy between ACT and DVE so
                # neither engine's per-op drain serializes the output phase
                if tb % 2 == 0:
                    nc.scalar.copy(ob[:], po[:])
                else:
                    nc.vector.tensor_copy(ob[:], po[:])
                nc.sync.dma_start(out[tb * P : (tb + 1) * P, :], ob[:])

    nc.compile()
    return nc


def get_program():
    if "nc" not in _CACHE:
        _CACHE["nc"] = _build_program()
    return _CACHE["nc"]


def make_in_maps(query, pos_emb, w_qkv, w_out):
    import ml_dtypes

    bf16 = ml_dtypes.bfloat16
    query = np.asarray(query, dtype=np.float32)
    pos_emb = np.asarray(pos_emb, dtype=np.float32)
    w_qkv = np.asarray(w_qkv, dtype=np.float32)
    w_out = np.asarray(w_out, dtype=np.float32)
    posT = np.ascontiguousarray(pos_emb.T).astype(bf16)
    in_maps = []
    for c in range(NCORES):
        b, hb = c // (NCORES // B), (c % (NCORES // B)) * HPC
        heads = range(hb, hb + HPC)
        # w_qkv column e for head h, kind j (q/k/v), dim d: e = h*3*DK + j*DK + d
        wq_c = np.concatenate(
            [w_qkv[:, h * 3 * DK + j * DK : h * 3 * DK + (j + 1) * DK] for j in range(3) for h in heads],
            axis=1,
        )
        wout_c = np.concatenate([w_out[h * DK : (h + 1) * DK, :] for h in heads], axis=0)
        in_maps.append(
            {
                "xqT": np.ascontiguousarray(query[b].T).astype(bf16),
                "posT": posT,
                "wqkv": np.ascontiguousarray(wq_c).astype(bf16),
                "wout": np.ascontiguousarray(wout_c),
            }
        )
    return in_maps


def gather_output(results):
    out = np.zeros((B, S, D), dtype=np.float32)
    for c in range(NCORES):
        out[c // (NCORES // B)] += np.asarray(results[c]["out"], dtype=np.float32)
    return out


def kernel(query, pos_emb, w_qkv, w_out):
    from concourse.bass_utils import run_bass_kernel_spmd

    nc = get_program()
    in_maps = make_in_maps(query, pos_emb, w_qkv, w_out)
    res = run_bass_kernel_spmd(nc, in_maps, list(range(NCORES)))
    return gather_output(res.results)


# revision 26
# speedup vs baseline: 1.0701x; 1.0145x over previous
"""Multi-headed self-attention (B=2, S=2048, D=1024, H=16) on 8 TRN2 cores.

Sharding: hybrid batch x head tensor-parallel. Core c handles batch c//4 and
heads (c%4)*4 .. (c%4)*4+3. Each core computes x = query[b] + pos_emb, the QKV
projection for its 4 heads, attention, and a partial output projection
(o_heads @ w_out_rows). Host sums the 4 partials per batch.

v2 notes (vs the f32r v1 baseline):
- Inputs (queryT, pos_embT, w_qkv) ship in bf16: halves the input DMA and the
  QKV projection runs as bf16 matmuls (same PE rate as f32r, exact f32 psum).
- The exp over attention scores is split between the Scalar engine (true exp)
  and the Vector engine (Schraudolph-style 2^y bit trick: i16 = s*C + B,
  bitcast to bf16), so the PE's QK->AV pipeline is never gated on a single
  activation engine. DVE handles DVE_KBS of the 16 key blocks per head.
- Attention weights (ptile) and V are bf16; QK^T consumes bf16 q/k. All psum
  accumulation stays f32.
- Softmax denominators come from a ones-column in the AV matmul; reciprocal is
  applied to the single denominator row, then broadcast to 64 partitions via a
  DRAM-staged partition-broadcast DMA (bandwidth is idle during attention).
- Output projection keeps f32r (oT x w_out), result stored bf16 (halves the
  output DMA); the host upcasts and sums partials in f32.
"""

import os
import sys

import numpy as np

if "/opt/trn_rl_repo" not in sys.path:
    sys.path.insert(0, "/opt/trn_rl_repo")

B, S, D, H = 2, 2048, 1024, 16
DK = 64
P = 128
NCORES = 8
HPC = H // (NCORES // B)  # heads per core = 4
T = S  # tokens per core (one batch)
E = HPC * 3 * DK  # 768 qkv output columns per core
NDC = D // P  # 8 contraction chunks
NEC = E // P  # 6 projection output chunks
NTB = T // P  # 16 token blocks
NTG = T // 512  # 4 token groups of 512
SCALE = DK**-0.5

# Schraudolph bf16-bits exp: exp(s*SCALE) ~ bf16(int16(s*EXP_C + EXP_B))
EXP_C = float(SCALE * 128 * np.log2(np.e))
EXP_B = 16249.1
# key blocks (of 16 per head) whose exp runs on DVE instead of ACT
DVE_SET = (3, 8, 13)
# timing probe: 1 = skip exp instructions (AV consumes garbage), 2 = skip AV
PROBE_SKIP = int(os.environ.get("PROBE_SKIP", "0"))

_CACHE = {}


def _build_program(reps=1):
    from contextlib import ExitStack, nullcontext

    import concourse.bass as bass
    import concourse.tile as tile
    from concourse import bacc
    from concourse import mybir
    from concourse.masks import make_identity

    f32 = mybir.dt.float32
    f32r = mybir.dt.float32r
    bf16 = mybir.dt.bfloat16
    i16 = mybir.dt.int16
    EXP = mybir.ActivationFunctionType.Exp
    MULT = mybir.AluOpType.mult
    ADD = mybir.AluOpType.add

    nc = bacc.Bacc()
    xqT = nc.declare_dram_parameter("xqT", [D, T], bf16, isOutput=False)
    posT = nc.declare_dram_parameter("posT", [D, T], bf16, isOutput=False)
    wqkv = nc.declare_dram_parameter("wqkv", [D, E], bf16, isOutput=False)
    wout = nc.declare_dram_parameter("wout", [HPC * DK, D], f32, isOutput=False)
    out = nc.declare_dram_parameter("out", [T, D], bf16, isOutput=True)

    with tile.TileContext(nc) as tc, ExitStack() as top:
        const = top.enter_context(tc.tile_pool(name="const", bufs=1))
        w_sb = const.tile([P, NDC, E], bf16)
        wout_sb = const.tile([P, 2, D], f32r)
        ident = const.tile([P, P], bf16)
        make_identity(nc, ident[:])
        qkvT = const.tile([P, 2, T], bf16)  # q01, q23 (feature-major)
        # Per-head k, zero-padded to the full 128 partitions: rows (h%2)*64..+64
        # hold k_h, the other 64 rows stay zero. QK^T then contracts over 128
        # partitions (the pad rows kill the co-resident head's q in the moving
        # operand), keeping the PE in the same (128,128) tile config as every
        # other matmul in the kernel -- no config switching in the kb loop.
        kTp = const.tile([P, HPC, T], bf16)
        V_sb = const.tile([P, NTB, HPC, DK + 2], bf16)  # col DK = ones
        oT = const.tile([P, 2, T], f32r)  # per-head-pair attention outputs

        for h in range(HPC):
            nc.gpsimd.memset(V_sb[:, :, h, DK : DK + 1], 1.0)
            pad0 = (1 - h % 2) * DK
            nc.gpsimd.memset(kTp[pad0 : pad0 + DK, h, :], 0.0)

        # reps>1 wraps the body in an on-device loop (timing builds only)
        rep_ctx = tc.For_i(0, reps, 1) if reps > 1 else nullcontext()
        top.enter_context(rep_ctx)

        # ---- Phase 1: x = queryT + pos_embT (bf16), qkv projection ----
        with (
            tc.tile_pool(name="xt", bufs=3) as xt_pool,
            tc.tile_pool(name="ldt", bufs=8) as ld_pool,
            tc.tile_pool(name="vt", bufs=3) as vt_pool,
            tc.tile_pool(name="psp", bufs=6, space="PSUM") as psum_p,
            tc.tile_pool(name="pst", bufs=2, space="PSUM") as psum_t,
        ):
            for tg in range(NTG):
                c0 = tg * 512
                xts = xt_pool.tile([P, NDC, 512], bf16, name="xts", tag="xt")
                ps = [
                    psum_p.tile([P, 512], f32, name=f"psp{ec}", tag="psp")
                    for ec in range(NEC)
                ]
                for dc in range(NDC):
                    if tg == 0:
                        nc.sync.dma_start(w_sb[:, dc, :], wqkv[dc * P : (dc + 1) * P, :])
                    qt = ld_pool.tile([P, 512], bf16, tag="ldtmp", name="qt")
                    nc.sync.dma_start(qt[:], xqT[dc * P : (dc + 1) * P, c0 : c0 + 512])
                    pt = ld_pool.tile([P, 512], bf16, tag="ldtmp", name="pt")
                    nc.sync.dma_start(pt[:], posT[dc * P : (dc + 1) * P, c0 : c0 + 512])
                    nc.vector.tensor_add(xts[:, dc, :], qt[:], pt[:])
                    for ec in range(NEC):
                        nc.tensor.matmul(
                            ps[ec][:],
                            w_sb[:, dc, ec * P : (ec + 1) * P],
                            xts[:, dc, :],
                            start=(dc == 0),
                            stop=(dc == NDC - 1),
                        )
                # q01,q23 -> qkvT; k01,k23 -> per-head padded kTp slots;
                # v01,v23 -> transpose into V_sb
                for ec in range(2):
                    nc.any.tensor_copy(qkvT[:, ec, c0 : c0 + 512], ps[ec][:])
                for ec in range(2, 4):
                    for half in range(2):
                        h = (ec - 2) * 2 + half
                        r0 = half * DK
                        nc.any.tensor_copy(
                            kTp[r0 : r0 + DK, h, c0 : c0 + 512],
                            ps[ec][r0 : r0 + DK, :],
                        )
                for evc in range(2):
                    vtmp = vt_pool.tile([P, 512], bf16, tag="vtmp", name="vtmp")
                    nc.any.tensor_copy(vtmp[:], ps[4 + evc][:])
                    for i in range(4):
                        tb = tg * 4 + i
                        pst = psum_t.tile([P, P], bf16, tag="pst", name="pst")
                        nc.tensor.transpose(
                            pst[:], vtmp[:, i * P : (i + 1) * P], ident[:]
                        )
                        nc.any.tensor_copy(
                            V_sb[:, tb, 2 * evc : 2 * evc + 2, 0:DK],
                            pst.rearrange("p (h d) -> p h d", h=2),
                        )

        # ---- Phase 2: attention per head ----
        with (
            tc.tile_pool(name="ptl", bufs=4) as pt_pool,
            tc.tile_pool(name="ou", bufs=2) as ou_pool,
            tc.tile_pool(name="rb", bufs=2) as rb_pool,
            tc.tile_pool(name="sdp", bufs=2, space="DRAM") as dram_pool,
            tc.tile_pool(name="psqk", bufs=2, space="PSUM") as psum_qk,
            tc.tile_pool(name="psav", bufs=1, space="PSUM") as psum_av,
        ):
            for h in range(HPC):
                ecq, row = h // 2, (h % 2) * DK
                qT = qkvT[:, ecq, :]  # full 128 rows; pad rows in kTp zero them
                kT = kTp[:, h, :]
                # AV accumulator split into two half-query tiles (2 banks each)
                # so the first half frees for the next head's AV as soon as its
                # norm copy lands, instead of after the full-width copy
                poTs = (
                    psum_av.tile([DK + 1, T // 2], f32, tag="pa", name="poTa"),
                    psum_av.tile([DK + 1, T // 2], f32, tag="pb", name="poTb"),
                )

                def emit_qk(kb, qT=qT, kT=kT):
                    # exp(scores^T * scale) for key block kb over all queries
                    ptile = pt_pool.tile([P, T], bf16, name="ptile", tag="pt")
                    for qh in range(2):
                        pqk = psum_qk.tile([P, 1024], f32, name="pqk", tag="pqk")
                        for qq in range(2):
                            q0 = qh * 1024 + qq * 512
                            nc.tensor.matmul(
                                pqk[:, qq * 512 : (qq + 1) * 512],
                                kT[:, kb * P : (kb + 1) * P],
                                qT[:, q0 : q0 + 512],
                                start=True,
                                stop=True,
                            )
                        dst = ptile[:, qh * 1024 : (qh + 1) * 1024]
                        if PROBE_SKIP == 1:
                            if qh == 0:
                                nc.vector.memset(ptile[:, 0:16], 1.0)
                        elif kb in DVE_SET:
                            nc.vector.tensor_scalar(
                                dst.bitcast(i16), pqk[:], EXP_C, EXP_B, MULT, ADD
                            )
                        else:
                            nc.scalar.activation(dst, pqk[:], EXP, scale=SCALE)
                    return ptile

                # software pipeline: QK(kb+1) issues on PE while ACT/DVE exps
                # kb's scores, so exp latency stays off the PE critical path
                ptile_cur = emit_qk(0)
                for kb in range(NTB):
                    ptile_next = emit_qk(kb + 1) if kb + 1 < NTB else None
                    for qg in range(NTG):
                        nc.tensor.matmul(
                            poTs[qg // 2][:, (qg % 2) * 512 : (qg % 2 + 1) * 512],
                            V_sb[:, kb, h, 0 : DK + 1],
                            ptile_cur[:, qg * 512 : (qg + 1) * 512],
                            start=(kb == 0),
                            stop=(kb == NTB - 1),
                        )
                    ptile_cur = ptile_next

                # Normalize: copy frees the AV psum; reciprocal on the single
                # denominator row, then broadcast to 64 partitions via DRAM
                # (attention leaves DMA bandwidth idle). For the last head the
                # whole chain is chunked per token group so the output
                # projection starts ~3us (not ~10us) after the last AV matmul.
                o_us = ou_pool.tile([DK + 1, T], f32, tag="ous")
                rbc = rb_pool.tile([DK, T], f32, tag="rbc")
                s_dram = dram_pool.tile([1, T], f32, name="sdram", tag="sd")
                # The whole chain stays on DVE: offloading the copy to ACT or
                # the muls to GpSimd both measured slower on HW (the chain is
                # latency-critical at head boundaries; cross-engine hops and
                # queueing behind ACT's exps cost more than DVE's drain).
                for pi, poX in enumerate(poTs):
                    base = pi * (T // 2)
                    chunks = 2 if h == HPC - 1 else 1
                    csz = (T // 2) // chunks
                    for cc in range(chunks):
                        l0 = cc * csz
                        sl = slice(base + l0, base + l0 + csz)
                        pl = slice(l0, l0 + csz)
                        nc.vector.tensor_copy(o_us[:, sl], poX[:, pl])
                        nc.vector.reciprocal(
                            o_us[DK : DK + 1, sl], o_us[DK : DK + 1, sl]
                        )
                        nc.sync.dma_start(s_dram[:, sl], o_us[DK : DK + 1, sl])
                        nc.sync.dma_start(
                            rbc[:, sl], s_dram[:, sl].partition_broadcast(DK)
                        )
                        for qg in range(csz // 512):
                            ql = slice(base + l0 + qg * 512, base + l0 + (qg + 1) * 512)
                            nc.vector.tensor_mul(
                                oT[row : row + DK, ecq, ql], o_us[0:DK, ql], rbc[:, ql]
                            )

        # ---- Phase 3: partial output projection ----
        nc.sync.dma_start(
            wout_sb[:], wout.rearrange("(c p) n -> p c n", p=P).bitcast(f32r)
        )
        with (
            tc.tile_pool(name="pso", bufs=2, space="PSUM") as psum_o,
            tc.tile_pool(name="osb", bufs=3) as osb_pool,
        ):
            for tb in range(NTB):
                po = psum_o.tile([P, D], f32)
                for pair in range(2):
                    for nh in range(2):
                        nc.tensor.matmul(
                            po[:, nh * 512 : (nh + 1) * 512],
                            oT[:, pair, tb * P : (tb + 1) * P],
                            wout_sb[:, pair, nh * 512 : (nh + 1) * 512],
                            start=(pair == 0),
                            stop=(pair == 1),
                        )
                ob = osb_pool.tile([P, D], bf16)
                # alternate the psum-draining copy between ACT and DVE so
                # neither engine's per-op drain serializes the output phase
                if tb % 2 == 0:
                    nc.scalar.copy(ob[:], po[:])
                else:
                    nc.vector.tensor_copy(ob[:], po[:])
                nc.sync.dma_start(out[tb * P : (tb + 1) * P, :], ob[:])

    nc.compile()
    return nc


def get_program():
    if "nc" not in _CACHE:
        _CACHE["nc"] = _build_program()
    return _CACHE["nc"]


def make_in_maps(query, pos_emb, w_qkv, w_out):
    import ml_dtypes

    bf16 = ml_dtypes.bfloat16
    query = np.asarray(query, dtype=np.float32)
    pos_emb = np.asarray(pos_emb, dtype=np.float32)
    w_qkv = np.asarray(w_qkv, dtype=np.float32)
    w_out = np.asarray(w_out, dtype=np.float32)
    posT = np.ascontiguousarray(pos_emb.T).astype(bf16)
    in_maps = []
    for c in range(NCORES):
        b, hb = c // (NCORES // B), (c % (NCORES // B)) * HPC
        heads = range(hb, hb + HPC)
        # w_qkv column e for head h, kind j (q/k/v), dim d: e = h*3*DK + j*DK + d
        wq_c = np.concatenate(
            [w_qkv[:, h * 3 * DK + j * DK : h * 3 * DK + (j + 1) * DK] for j in range(3) for h in heads],
            axis=1,
        )
        wout_c = np.concatenate([w_out[h * DK : (h + 1) * DK, :] for h in heads], axis=0)
        in_maps.append(
            {
                "xqT": np.ascontiguousarray(query[b].T).astype(bf16),
                "posT": posT,
                "wqkv": np.ascontiguousarray(wq_c).astype(bf16),
                "wout": np.ascontiguousarray(wout_c),
            }
        )
    return in_maps


def gather_output(results):
    out = np.zeros((B, S, D), dtype=np.float32)
    for c in range(NCORES):
        out[c // (NCORES // B)] += np.asarray(results[c]["out"], dtype=np.float32)
    return out


def kernel(query, pos_emb, w_qkv, w_out):
    from concourse.bass_utils import run_bass_kernel_spmd

    nc = get_program()
    in_maps = make_in_maps(query, pos_emb, w_qkv, w_out)
    res = run_bass_kernel_spmd(nc, in_maps, list(range(NCORES)))
    return gather_output(res.results)
